# revision 45
# baseline (speedup 1.0000x reference)
"""Multi-head attention (B=2, N=4096, C=512, H=8) on 8 trn2 NeuronCores.

Sharding: core -> (batch b = core//4, head-pair hp = core%4), i.e. data
parallel over B and tensor parallel over the 8 heads (2 heads per core),
with column-sharded qkv weights and row-sharded proj weights. Each core
returns a partial projection output [4096, 512]; the host sums the 4
head-pair partials per batch and adds proj_b.

Per-core device kernel (flash-style, nothing N^2 ever hits HBM):
  qT/kT  [128(=2 heads x 64 feat), 4096]  <- wqk^T @ x^T   (bf16 matmuls)
  v_sb   [128 keys, 32 m-tiles, 65]       <- x^T^T @ wv (+bias), ones col
  per (query-group g of 512 queries, head h), chunks of CH=3 key m-tiles:
    S^T chunk [128 keys, 3*512 q] in PSUM <- kT_m-x-qT  (scores matmuls)
    E = exp(SCALE * S^T) on the ACT engine -> SBUF bf16 (one ACTIVATE/chunk)
    out^T [65, 512] PSUM += v_aug-x-E  (row 64 = softmax denominator, free
    via the ones column; accumulation emitted SKEW chunks behind scores)
  per (g, h) tail (deferred via a pending queue, one piece per chunk, so
  PE/ACT stay co-saturated and the HAM clock never throttles):
    drain out^T to SBUF; transpose denom row to [128, 4] with N=1 matmuls;
    128-lane reciprocal; per-head proj of the UNNORMALIZED out^T; scale the
    proj psum per-partition (token) by 1/denom on DVE; sum heads; DMA out.
"""

import numpy as np

_state = {}

B, N, C, H, DH = 2, 4096, 512, 8, 64
SCALE = DH ** -0.5
GQ = 512          # queries per group
NG = N // GQ      # 8 groups
MT = N // 128     # 32 key m-tiles
CH = 3            # m-tiles per exp chunk


def _build_nc(debug=False):
    from contextlib import ExitStack

    import concourse.bacc as bacc
    import concourse.tile as tile
    from concourse import mybir

    bf16 = mybir.dt.bfloat16
    f32 = mybir.dt.float32
    f32r = mybir.dt.float32r
    EXP = mybir.ActivationFunctionType.Exp

    nc = bacc.Bacc(None, target_bir_lowering=False)
    with tile.TileContext(nc) as tc, ExitStack() as ctx:
        dram = ctx.enter_context(tc.tile_pool(name="dram", bufs=1, space="DRAM"))
        xt_d = dram.tile([C, N], bf16, kind="ExternalInput", name="xt",
                         uniquify=False, tag="dxt")
        wqk_d = dram.tile([C, 256], bf16, kind="ExternalInput", name="wqk",
                          uniquify=False, tag="dwqk")
        bqk_d = dram.tile([128, 2], f32, kind="ExternalInput", name="bqk",
                          uniquify=False, tag="dbqk")
        wv_d = dram.tile([C, 128], bf16, kind="ExternalInput", name="wv",
                         uniquify=False, tag="dwv")
        bv_d = dram.tile([128, 128], bf16, kind="ExternalInput", name="bv",
                         uniquify=False, tag="dbv")
        pw_d = dram.tile([64, 1024], bf16, kind="ExternalInput", name="pw2",
                         uniquify=False, tag="dpw")
        out_d = dram.tile([N, C], f32, kind="ExternalOutput", name="out",
                          uniquify=False, tag="dout")
        if debug:
            dbg_qT = dram.tile([128, N], bf16, kind="ExternalOutput",
                               name="dbg_qT", uniquify=False, tag="dbg_qT")
            dbg_kT = dram.tile([128, N], bf16, kind="ExternalOutput",
                               name="dbg_kT", uniquify=False, tag="dbg_kT")
            dbg_v = dram.tile([128, MT, 130], bf16, kind="ExternalOutput",
                              name="dbg_v", uniquify=False, tag="dbg_v")
            dbg_e = dram.tile([128, 3 * GQ], bf16, kind="ExternalOutput",
                              name="dbg_e", uniquify=False, tag="dbg_e")
            dbg_at = dram.tile([64, GQ], bf16, kind="ExternalOutput",
                               name="dbg_at", uniquify=False, tag="dbg_at")
            dbg_rb = dram.tile([1, GQ], bf16, kind="ExternalOutput",
                               name="dbg_rb", uniquify=False, tag="dbg_rb")

        const = ctx.enter_context(tc.tile_pool(name="const", bufs=1))
        wqk_sb = const.tile([128, 4, 256], bf16, name="wqk_sb", tag="wqk_sb")
        wqk_r = wqk_d.rearrange("(k p) f -> p k f", p=128)
        for k in range(4):
            nc.gpsimd.dma_start(wqk_sb[:, k, :], wqk_r[:, k, :])
        wv_sb = const.tile([128, 4, 128], bf16, name="wv_sb", tag="wv_sb")
        wv_r = wv_d.rearrange("(k p) f -> p k f", p=128)
        for k in range(4):
            nc.gpsimd.dma_start(wv_sb[:, k, :], wv_r[:, k, :])
        bqk_sb = const.tile([128, 2], f32, name="bqk_sb", tag="bqk_sb")
        nc.gpsimd.dma_start(bqk_sb[:], bqk_d[:])
        bv_sb = const.tile([128, 128], bf16, name="bv_sb", tag="bv_sb")
        nc.gpsimd.dma_start(bv_sb[:], bv_d[:])
        pw_sb = const.tile([64, 1024], bf16, name="pw_sb", tag="pw_sb")
        nc.gpsimd.dma_start(pw_sb[:], pw_d[:])
        ones_sb = const.tile([65, 128], bf16, name="ones_sb", tag="ones_sb")
        nc.vector.memset(ones_sb[:], 1.0)

        persist = ctx.enter_context(tc.tile_pool(name="persist", bufs=1))
        qT = persist.tile([128, N], bf16, name="qT", tag="qT")
        kT = persist.tile([128, N], bf16, name="kT", tag="kT")
        vsb = persist.tile([128, MT, 130], bf16, name="vsb", tag="vsb")
        vones = vsb.rearrange("p m (a b) -> p m a b", a=2)
        nc.vector.memset(vones[:, :, 0, 64:65], 1.0)
        nc.vector.memset(vones[:, :, 1, 64:65], 1.0)

        xpool = ctx.enter_context(tc.tile_pool(name="xp", bufs=4))
        spool = ctx.enter_context(tc.tile_pool(name="sp", bufs=2, space="PSUM"))
        apool = ctx.enter_context(tc.tile_pool(name="ap", bufs=2, space="PSUM"))
        epool = ctx.enter_context(tc.tile_pool(name="ep", bufs=6))
        rpool = ctx.enter_context(tc.tile_pool(name="rp", bufs=2))
        opool = ctx.enter_context(tc.tile_pool(name="op", bufs=3))

        xt_r = xt_d.rearrange("(k p) n -> p k n", p=128)

        next_qk = [0]
        next_v = [0]
        xtiles = {}

        def emit_qk(g):
            xtile = xpool.tile([128, 4, GQ], bf16, name="xtile", tag="xtile")
            xtiles[g] = xtile
            for k in range(4):
                nc.sync.dma_start(xtile[:, k, :],
                                  xt_r[:, k, GQ * g:GQ * (g + 1)])
            qkp = spool.tile([128, 3 * GQ], f32, name="qkp", tag="sch")
            for k in range(4):
                nc.tensor.matmul(qkp[:, 0:512], wqk_sb[:, k, 0:128],
                                 xtile[:, k, :], start=(k == 0), stop=(k == 3))
            for k in range(4):
                nc.tensor.matmul(qkp[:, 512:1024], wqk_sb[:, k, 128:256],
                                 xtile[:, k, :], start=(k == 0), stop=(k == 3))
            nc.vector.tensor_scalar_add(qT[:, GQ * g:GQ * (g + 1)],
                                        qkp[:, 0:512], bqk_sb[:, 0:1])
            nc.vector.tensor_scalar_add(kT[:, GQ * g:GQ * (g + 1)],
                                        qkp[:, 512:1024], bqk_sb[:, 1:2])

        def emit_v(m):
            g, t = divmod(m, 4)
            xtile = xtiles[g]
            vp = apool.tile([128, 512], f32, name="vp", tag="av")
            for k in range(4):
                nc.tensor.matmul(vp[:, 0:128],
                                 xtile[:, k, 128 * t:128 * (t + 1)],
                                 wv_sb[:, k, :],
                                 start=(k == 0), stop=(k == 3))
            src = vp[:, 0:128].rearrange("p (a b) -> p a b", a=2)
            dst = vsb[:, m, :].rearrange("p (a b) -> p a b", a=2)
            bvv = bv_sb.rearrange("p (a b) -> p a b", a=2)
            nc.vector.tensor_add(dst[:, :, 0:64], src, bvv)

        def need_qk(gq):
            while next_qk[0] <= gq:
                emit_qk(next_qk[0])
                next_qk[0] += 1

        def need_v(m):
            while next_v[0] <= m:
                need_qk(next_v[0] // 4)
                emit_v(next_v[0])
                next_v[0] += 1

        chunks = [list(range(c, min(c + CH, MT))) for c in range(0, MT, CH)]
        items = [(g, h, ms) for g in range(NG) for h in (0, 1) for ms in chunks]

        av_tiles = {}
        at_tiles = {}
        proj_boxes = {}
        pending = []

        def emit_scores(g, h, ms):
            need_qk(max(ms[-1] // 4, g))
            st = spool.tile([128, 3 * GQ], f32, name="st", tag="sch")
            for j, m in enumerate(ms):
                nc.tensor.matmul(st[:, 512 * j:512 * (j + 1)],
                                 kT[64 * h:64 * h + 64, 128 * m:128 * (m + 1)],
                                 qT[64 * h:64 * h + 64, GQ * g:GQ * (g + 1)],
                                 start=True, stop=True)
            et = epool.tile([128, 3 * GQ], bf16, name="et", tag="et")
            w = 512 * len(ms)
            nc.scalar.activation(et[:, 0:w], st[:, 0:w], EXP, scale=SCALE)
            if pending:
                pending.pop(0)()
            if debug and g == 0 and h == 0 and ms[0] == 0:
                nc.sync.dma_start(dbg_e[:], et[:])
            return et

        def emit_post(g, h):
            a = av_tiles.pop((g, h))
            # drain AV psum to SBUF right away: frees the psum slot for the
            # next (g, h) accumulation without waiting on the reciprocal.
            asb = rpool.tile([65, 512], bf16, name="asb", tag="asb")
            nc.vector.tensor_copy(asb[:], a[0:65, :])
            # transpose denominator row [1, 512] -> [128, 4] via N=1 matmuls
            # so the reciprocal runs on 128 lanes instead of 1.
            rtp = apool.tile([128, 512], f32, name="rtp", tag="av")
            for t in range(4):
                nc.tensor.matmul(rtp[:, t:t + 1],
                                 asb[64:65, 128 * t:128 * (t + 1)],
                                 ones_sb[64:65, 0:1], start=True, stop=True)
            rts = rpool.tile([128, 4], f32, name="rts", tag="rts")
            nc.vector.reciprocal(rts[:], rtp[:, 0:4])
            at_tiles[(g, h)] = (asb, rts)
            if debug and g == 0 and h == 0:
                nc.sync.dma_start(dbg_at[:], asb[0:64, :])
            box = proj_boxes.setdefault(g, {})
            a_, r_ = asb, rts
            for t in range(4):
                if g == NG - 1 and h == 1:
                    proj_h1(g, a_, r_, t, box, act=True)
                elif h == 0:
                    pending.append(lambda t=t: proj_h0(g, a_, r_, t, box))
                else:
                    pending.append(lambda t=t: proj_h1(g, a_, r_, t, box))

        def proj_h0(g, a0, r0, t, box, act=False):
            pp0 = apool.tile([128, 512], f32, name="pp0", tag="av")
            nc.tensor.matmul(pp0[:], a0[0:64, 128 * t:128 * (t + 1)],
                             pw_sb[0:64, 0:512], start=True, stop=True)
            t0 = opool.tile([128, 512], bf16, name="t0", tag="t0")
            if act:
                nc.scalar.mul(t0[:], pp0[:], r0[:, t:t + 1])
            else:
                nc.vector.tensor_scalar_mul(t0[:], pp0[:], r0[:, t:t + 1])
            box[t] = t0

        def proj_h1(g, a1, r1, t, box, act=False):
            pp1 = apool.tile([128, 512], f32, name="pp1", tag="av")
            nc.tensor.matmul(pp1[:], a1[0:64, 128 * t:128 * (t + 1)],
                             pw_sb[0:64, 512:1024], start=True, stop=True)
            t1 = opool.tile([128, 512], bf16, name="t1", tag="t1")
            if act:
                nc.scalar.mul(t1[:], pp1[:], r1[:, t:t + 1])
            else:
                nc.vector.tensor_scalar_mul(t1[:], pp1[:], r1[:, t:t + 1])
            po = opool.tile([128, 512], f32, name="po", tag="po")
            nc.vector.tensor_add(po[:], box.pop(t), t1[:])
            dma = nc.gpsimd.dma_start if t % 2 else nc.sync.dma_start
            dma(out_d[GQ * g + 128 * t:GQ * g + 128 * (t + 1), :], po[:])

        def emit_av(g, h, ms, et):
            if (g, h) not in av_tiles:
                av_tiles[(g, h)] = apool.tile([128, 512], f32, name="avt",
                                              tag="av")
            a = av_tiles[(g, h)]
            need_v(ms[-1])
            for j, m in enumerate(ms):
                nc.tensor.matmul(a[0:65, :], vsb[:, m, 65 * h:65 * h + 65],
                                 et[:, 512 * j:512 * (j + 1)],
                                 start=(m == 0), stop=(m == MT - 1),
                                 skip_group_check=True)
            if ms[-1] == MT - 1:
                emit_post(g, h)

        from collections import deque
        inflight = deque()
        SKEW = 4
        for it in items:
            et = emit_scores(*it)
            inflight.append((it, et))
            if len(inflight) > SKEW:
                (pg, ph, pms), pet = inflight.popleft()
                emit_av(pg, ph, pms, pet)
        while inflight:
            (pg, ph, pms), pet = inflight.popleft()
            emit_av(pg, ph, pms, pet)
        while pending:
            pending.pop(0)()

        if debug:
            nc.sync.dma_start(dbg_qT[:], qT[:])
            nc.sync.dma_start(dbg_kT[:], kT[:])
            nc.sync.dma_start(dbg_v[:], vsb[:])

    nc.compile()
    return nc


def _get_nc():
    if "nc" not in _state:
        _state["nc"] = _build_nc()
    return _state["nc"]


def _make_in_maps(x, qkv_w, qkv_b, proj_w):
    import ml_dtypes
    bf = ml_dtypes.bfloat16
    x = np.asarray(x, np.float32)
    qkv_w = np.asarray(qkv_w, np.float32)
    qkv_b = np.asarray(qkv_b, np.float32)
    proj_w = np.asarray(proj_w, np.float32)
    in_maps = []
    for core in range(8):
        b, hp = divmod(core, 4)
        h0, h1 = 2 * hp, 2 * hp + 1
        xt = np.ascontiguousarray(x[b].T).astype(bf)
        rq = np.concatenate([qkv_w[64 * h0:64 * h0 + 64],
                             qkv_w[64 * h1:64 * h1 + 64]], 0)
        rk = np.concatenate([qkv_w[C + 64 * h0:C + 64 * h0 + 64],
                             qkv_w[C + 64 * h1:C + 64 * h1 + 64]], 0)
        wqk = np.ascontiguousarray(np.concatenate([rq, rk], 0).T).astype(bf)
        bq = np.concatenate([qkv_b[64 * h0:64 * h0 + 64],
                             qkv_b[64 * h1:64 * h1 + 64]])
        bk = np.concatenate([qkv_b[C + 64 * h0:C + 64 * h0 + 64],
                             qkv_b[C + 64 * h1:C + 64 * h1 + 64]])
        bqk = np.ascontiguousarray(np.stack([bq, bk], 1)).astype(np.float32)
        rv = np.concatenate([qkv_w[2 * C + 64 * h0:2 * C + 64 * h0 + 64],
                             qkv_w[2 * C + 64 * h1:2 * C + 64 * h1 + 64]], 0)
        wv = np.ascontiguousarray(rv.T).astype(bf)
        bvrow = np.concatenate([qkv_b[2 * C + 64 * h0:2 * C + 64 * h0 + 64],
                                qkv_b[2 * C + 64 * h1:2 * C + 64 * h1 + 64]])
        bv = np.ascontiguousarray(
            np.broadcast_to(bvrow[None, :], (128, 128))).astype(bf)
        pwT = np.ascontiguousarray(proj_w[:, 128 * hp:128 * hp + 128].T)
        pw2 = np.ascontiguousarray(
            np.concatenate([pwT[0:64], pwT[64:128]], 1)).astype(bf)
        in_maps.append(dict(xt=xt, wqk=wqk, bqk=bqk, wv=wv, bv=bv, pw2=pw2))
    return in_maps


def _gather(results, proj_b):
    proj_b = np.asarray(proj_b, np.float32)
    out = np.empty((B, N, C), np.float32)
    for b in range(B):
        acc = results[4 * b]["out"].astype(np.float32).copy()
        for hp in range(1, 4):
            acc += results[4 * b + hp]["out"]
        out[b] = acc + proj_b[None, :]
    return out


def _run(x, qkv_w, qkv_b, proj_w, proj_b, trace=False, tmpdir=None):
    from concourse import bass_utils
    nc = _get_nc()
    in_maps = _make_in_maps(x, qkv_w, qkv_b, proj_w)
    res = bass_utils.run_bass_kernel_spmd(
        nc, in_maps, core_ids=list(range(8)), trace=trace, tmpdir=tmpdir)
    return _gather(res.results, proj_b), res


def kernel(x, qkv_w, qkv_b, proj_w, proj_b):
    out, _ = _run(x, qkv_w, qkv_b, proj_w, proj_b, trace=False)
    return out


# revision 47
# speedup vs baseline: 1.0149x; 1.0149x over previous
"""Multi-head attention (B=2, N=4096, C=512, H=8) on 8 trn2 NeuronCores.

Sharding: core -> (batch b = core//4, head-pair hp = core%4), i.e. data
parallel over B and tensor parallel over the 8 heads (2 heads per core),
with column-sharded qkv weights and row-sharded proj weights. Each core
returns a partial projection output [4096, 512]; the host sums the 4
head-pair partials per batch and adds proj_b.

Per-core device kernel (flash-style, nothing N^2 ever hits HBM):
  qT/kT  [128(=2 heads x 64 feat), 4096]  <- wqk^T @ x^T   (bf16 matmuls)
  v_sb   [128 keys, 32 m-tiles, 65]       <- x^T^T @ wv (+bias), ones col
  per (query-group g of 512 queries, head h), chunks of CH=3 key m-tiles:
    S^T chunk [128 keys, 3*512 q] in PSUM <- kT_m-x-qT  (scores matmuls)
    E = exp(SCALE * S^T) on the ACT engine -> SBUF bf16 (one ACTIVATE/chunk)
    out^T [65, 512] PSUM += v_aug-x-E  (row 64 = softmax denominator, free
    via the ones column; accumulation emitted SKEW chunks behind scores)
  per (g, h) tail (deferred via a pending queue, one piece per chunk, so
  PE/ACT stay co-saturated and the HAM clock never throttles):
    drain out^T to SBUF; transpose denom row to [128, 4] with N=1 matmuls;
    128-lane reciprocal; per-head proj of the UNNORMALIZED out^T; scale the
    proj psum per-partition (token) by 1/denom on DVE; sum heads; DMA out.
"""

import numpy as np

_state = {}

B, N, C, H, DH = 2, 4096, 512, 8, 64
SCALE = DH ** -0.5
GQ = 512          # queries per group
NG = N // GQ      # 8 groups
MT = N // 128     # 32 key m-tiles
CH = 3            # m-tiles per exp chunk


def _build_nc(debug=False):
    from contextlib import ExitStack

    import concourse.bacc as bacc
    import concourse.tile as tile
    from concourse import mybir

    bf16 = mybir.dt.bfloat16
    f32 = mybir.dt.float32
    f32r = mybir.dt.float32r
    EXP = mybir.ActivationFunctionType.Exp

    nc = bacc.Bacc(None, target_bir_lowering=False)
    with tile.TileContext(nc) as tc, ExitStack() as ctx:
        dram = ctx.enter_context(tc.tile_pool(name="dram", bufs=1, space="DRAM"))
        xt_d = dram.tile([C, N], bf16, kind="ExternalInput", name="xt",
                         uniquify=False, tag="dxt")
        wqk_d = dram.tile([C, 256], bf16, kind="ExternalInput", name="wqk",
                          uniquify=False, tag="dwqk")
        bqk_d = dram.tile([128, 2], f32, kind="ExternalInput", name="bqk",
                          uniquify=False, tag="dbqk")
        wv_d = dram.tile([C, 128], bf16, kind="ExternalInput", name="wv",
                         uniquify=False, tag="dwv")
        bv_d = dram.tile([128, 128], bf16, kind="ExternalInput", name="bv",
                         uniquify=False, tag="dbv")
        pw_d = dram.tile([64, 1024], bf16, kind="ExternalInput", name="pw2",
                         uniquify=False, tag="dpw")
        out_d = dram.tile([N, C], f32, kind="ExternalOutput", name="out",
                          uniquify=False, tag="dout")
        if debug:
            dbg_qT = dram.tile([128, N], bf16, kind="ExternalOutput",
                               name="dbg_qT", uniquify=False, tag="dbg_qT")
            dbg_kT = dram.tile([128, N], bf16, kind="ExternalOutput",
                               name="dbg_kT", uniquify=False, tag="dbg_kT")
            dbg_v = dram.tile([128, MT, 130], bf16, kind="ExternalOutput",
                              name="dbg_v", uniquify=False, tag="dbg_v")
            dbg_e = dram.tile([128, 3 * GQ], bf16, kind="ExternalOutput",
                              name="dbg_e", uniquify=False, tag="dbg_e")
            dbg_at = dram.tile([64, GQ], bf16, kind="ExternalOutput",
                               name="dbg_at", uniquify=False, tag="dbg_at")
            dbg_rb = dram.tile([1, GQ], bf16, kind="ExternalOutput",
                               name="dbg_rb", uniquify=False, tag="dbg_rb")

        const = ctx.enter_context(tc.tile_pool(name="const", bufs=1))
        wqk_sb = const.tile([128, 4, 256], bf16, name="wqk_sb", tag="wqk_sb")
        nc.gpsimd.dma_start(wqk_sb[:], wqk_d.rearrange("(k p) f -> p k f", p=128))
        wv_sb = const.tile([128, 4, 128], bf16, name="wv_sb", tag="wv_sb")
        nc.gpsimd.dma_start(wv_sb[:], wv_d.rearrange("(k p) f -> p k f", p=128))
        bqk_sb = const.tile([128, 2], f32, name="bqk_sb", tag="bqk_sb")
        nc.gpsimd.dma_start(bqk_sb[:], bqk_d[:])
        bv_sb = const.tile([128, 128], bf16, name="bv_sb", tag="bv_sb")
        nc.gpsimd.dma_start(bv_sb[:], bv_d[:])
        pw_sb = const.tile([64, 1024], bf16, name="pw_sb", tag="pw_sb")
        nc.gpsimd.dma_start(pw_sb[:], pw_d[:])
        ones_sb = const.tile([65, 128], bf16, name="ones_sb", tag="ones_sb")
        nc.vector.memset(ones_sb[:], 1.0)

        persist = ctx.enter_context(tc.tile_pool(name="persist", bufs=1))
        qT = persist.tile([128, N], bf16, name="qT", tag="qT")
        kT = persist.tile([128, N], bf16, name="kT", tag="kT")
        vsb = persist.tile([128, MT, 130], bf16, name="vsb", tag="vsb")
        vones = vsb.rearrange("p m (a b) -> p m a b", a=2)
        nc.vector.memset(vones[:, :, 0, 64:65], 1.0)
        nc.vector.memset(vones[:, :, 1, 64:65], 1.0)

        xpool = ctx.enter_context(tc.tile_pool(name="xp", bufs=4))
        spool = ctx.enter_context(tc.tile_pool(name="sp", bufs=2, space="PSUM"))
        apool = ctx.enter_context(tc.tile_pool(name="ap", bufs=2, space="PSUM"))
        epool = ctx.enter_context(tc.tile_pool(name="ep", bufs=6))
        rpool = ctx.enter_context(tc.tile_pool(name="rp", bufs=2))
        opool = ctx.enter_context(tc.tile_pool(name="op", bufs=3))

        xt_r = xt_d.rearrange("(k p) n -> p k n", p=128)

        next_qk = [0]
        next_v = [0]
        xtiles = {}

        def emit_qk(g):
            xtile = xpool.tile([128, 4, GQ], bf16, name="xtile", tag="xtile")
            xtiles[g] = xtile
            for k in range(4):
                nc.sync.dma_start(xtile[:, k, :],
                                  xt_r[:, k, GQ * g:GQ * (g + 1)])
            qkp = spool.tile([128, 3 * GQ], f32, name="qkp", tag="sch")
            for k in range(4):
                nc.tensor.matmul(qkp[:, 0:512], wqk_sb[:, k, 0:128],
                                 xtile[:, k, :], start=(k == 0), stop=(k == 3))
            for k in range(4):
                nc.tensor.matmul(qkp[:, 512:1024], wqk_sb[:, k, 128:256],
                                 xtile[:, k, :], start=(k == 0), stop=(k == 3))
            nc.vector.tensor_scalar_add(qT[:, GQ * g:GQ * (g + 1)],
                                        qkp[:, 0:512], bqk_sb[:, 0:1])
            nc.vector.tensor_scalar_add(kT[:, GQ * g:GQ * (g + 1)],
                                        qkp[:, 512:1024], bqk_sb[:, 1:2])

        def emit_v(m):
            g, t = divmod(m, 4)
            xtile = xtiles[g]
            vp = apool.tile([128, 512], f32, name="vp", tag="av")
            for k in range(4):
                nc.tensor.matmul(vp[:, 0:128],
                                 xtile[:, k, 128 * t:128 * (t + 1)],
                                 wv_sb[:, k, :],
                                 start=(k == 0), stop=(k == 3))
            src = vp[:, 0:128].rearrange("p (a b) -> p a b", a=2)
            dst = vsb[:, m, :].rearrange("p (a b) -> p a b", a=2)
            bvv = bv_sb.rearrange("p (a b) -> p a b", a=2)
            nc.vector.tensor_add(dst[:, :, 0:64], src, bvv)

        def need_qk(gq):
            while next_qk[0] <= gq:
                emit_qk(next_qk[0])
                next_qk[0] += 1

        def need_v(m):
            while next_v[0] <= m:
                need_qk(next_v[0] // 4)
                emit_v(next_v[0])
                next_v[0] += 1

        chunks = [list(range(c, min(c + CH, MT))) for c in range(0, MT, CH)]
        items = [(g, h, ms) for g in range(NG) for h in (0, 1) for ms in chunks]

        av_tiles = {}
        at_tiles = {}
        proj_boxes = {}
        pending = []

        def emit_scores(g, h, ms):
            need_qk(max(ms[-1] // 4, g))
            st = spool.tile([128, 3 * GQ], f32, name="st", tag="sch")
            for j, m in enumerate(ms):
                nc.tensor.matmul(st[:, 512 * j:512 * (j + 1)],
                                 kT[64 * h:64 * h + 64, 128 * m:128 * (m + 1)],
                                 qT[64 * h:64 * h + 64, GQ * g:GQ * (g + 1)],
                                 start=True, stop=True)
            et = epool.tile([128, 3 * GQ], bf16, name="et", tag="et")
            w = 512 * len(ms)
            nc.scalar.activation(et[:, 0:w], st[:, 0:w], EXP, scale=SCALE)
            if pending:
                pending.pop(0)()
            if debug and g == 0 and h == 0 and ms[0] == 0:
                nc.sync.dma_start(dbg_e[:], et[:])
            return et

        def emit_post(g, h):
            a = av_tiles.pop((g, h))
            # drain AV psum to SBUF right away: frees the psum slot for the
            # next (g, h) accumulation without waiting on the reciprocal.
            asb = rpool.tile([65, 512], bf16, name="asb", tag="asb")
            nc.vector.tensor_copy(asb[:], a[0:65, :])
            # transpose denominator row [1, 512] -> [128, 4] via N=1 matmuls
            # so the reciprocal runs on 128 lanes instead of 1.
            rtp = apool.tile([128, 512], f32, name="rtp", tag="av")
            for t in range(4):
                nc.tensor.matmul(rtp[:, t:t + 1],
                                 asb[64:65, 128 * t:128 * (t + 1)],
                                 ones_sb[64:65, 0:1], start=True, stop=True)
            rts = rpool.tile([128, 4], f32, name="rts", tag="rts")
            nc.vector.reciprocal(rts[:], rtp[:, 0:4])
            at_tiles[(g, h)] = (asb, rts)
            if debug and g == 0 and h == 0:
                nc.sync.dma_start(dbg_at[:], asb[0:64, :])
            box = proj_boxes.setdefault(g, {})
            a_, r_ = asb, rts
            for t in range(4):
                if g == NG - 1 and h == 1:
                    proj_h1(g, a_, r_, t, box, act=True)
                elif h == 0:
                    pending.append(lambda t=t: proj_h0(g, a_, r_, t, box))
                else:
                    pending.append(lambda t=t: proj_h1(g, a_, r_, t, box))

        def proj_h0(g, a0, r0, t, box, act=False):
            pp0 = apool.tile([128, 512], f32, name="pp0", tag="av")
            nc.tensor.matmul(pp0[:], a0[0:64, 128 * t:128 * (t + 1)],
                             pw_sb[0:64, 0:512], start=True, stop=True)
            t0 = opool.tile([128, 512], bf16, name="t0", tag="t0")
            if act:
                nc.scalar.mul(t0[:], pp0[:], r0[:, t:t + 1])
            else:
                nc.vector.tensor_scalar_mul(t0[:], pp0[:], r0[:, t:t + 1])
            box[t] = t0

        def proj_h1(g, a1, r1, t, box, act=False):
            pp1 = apool.tile([128, 512], f32, name="pp1", tag="av")
            nc.tensor.matmul(pp1[:], a1[0:64, 128 * t:128 * (t + 1)],
                             pw_sb[0:64, 512:1024], start=True, stop=True)
            t1 = opool.tile([128, 512], bf16, name="t1", tag="t1")
            if act:
                nc.scalar.mul(t1[:], pp1[:], r1[:, t:t + 1])
            else:
                nc.vector.tensor_scalar_mul(t1[:], pp1[:], r1[:, t:t + 1])
            po = opool.tile([128, 512], f32, name="po", tag="po")
            nc.vector.tensor_add(po[:], box.pop(t), t1[:])
            dma = nc.gpsimd.dma_start if t % 2 else nc.sync.dma_start
            dma(out_d[GQ * g + 128 * t:GQ * g + 128 * (t + 1), :], po[:])

        def emit_av(g, h, ms, et):
            if (g, h) not in av_tiles:
                av_tiles[(g, h)] = apool.tile([128, 512], f32, name="avt",
                                              tag="av")
            a = av_tiles[(g, h)]
            need_v(ms[-1])
            for j, m in enumerate(ms):
                nc.tensor.matmul(a[0:65, :], vsb[:, m, 65 * h:65 * h + 65],
                                 et[:, 512 * j:512 * (j + 1)],
                                 start=(m == 0), stop=(m == MT - 1),
                                 skip_group_check=True)
            if ms[-1] == MT - 1:
                emit_post(g, h)

        from collections import deque
        inflight = deque()
        SKEW = 4
        for it in items:
            et = emit_scores(*it)
            inflight.append((it, et))
            if len(inflight) > SKEW:
                (pg, ph, pms), pet = inflight.popleft()
                emit_av(pg, ph, pms, pet)
        while inflight:
            (pg, ph, pms), pet = inflight.popleft()
            emit_av(pg, ph, pms, pet)
        while pending:
            pending.pop(0)()

        if debug:
            nc.sync.dma_start(dbg_qT[:], qT[:])
            nc.sync.dma_start(dbg_kT[:], kT[:])
            nc.sync.dma_start(dbg_v[:], vsb[:])

    nc.compile()
    return nc


def _get_nc():
    if "nc" not in _state:
        _state["nc"] = _build_nc()
    return _state["nc"]


def _make_in_maps(x, qkv_w, qkv_b, proj_w):
    import ml_dtypes
    bf = ml_dtypes.bfloat16
    x = np.asarray(x, np.float32)
    qkv_w = np.asarray(qkv_w, np.float32)
    qkv_b = np.asarray(qkv_b, np.float32)
    proj_w = np.asarray(proj_w, np.float32)
    in_maps = []
    for core in range(8):
        b, hp = divmod(core, 4)
        h0, h1 = 2 * hp, 2 * hp + 1
        xt = np.ascontiguousarray(x[b].T).astype(bf)
        rq = np.concatenate([qkv_w[64 * h0:64 * h0 + 64],
                             qkv_w[64 * h1:64 * h1 + 64]], 0)
        rk = np.concatenate([qkv_w[C + 64 * h0:C + 64 * h0 + 64],
                             qkv_w[C + 64 * h1:C + 64 * h1 + 64]], 0)
        wqk = np.ascontiguousarray(np.concatenate([rq, rk], 0).T).astype(bf)
        bq = np.concatenate([qkv_b[64 * h0:64 * h0 + 64],
                             qkv_b[64 * h1:64 * h1 + 64]])
        bk = np.concatenate([qkv_b[C + 64 * h0:C + 64 * h0 + 64],
                             qkv_b[C + 64 * h1:C + 64 * h1 + 64]])
        bqk = np.ascontiguousarray(np.stack([bq, bk], 1)).astype(np.float32)
        rv = np.concatenate([qkv_w[2 * C + 64 * h0:2 * C + 64 * h0 + 64],
                             qkv_w[2 * C + 64 * h1:2 * C + 64 * h1 + 64]], 0)
        wv = np.ascontiguousarray(rv.T).astype(bf)
        bvrow = np.concatenate([qkv_b[2 * C + 64 * h0:2 * C + 64 * h0 + 64],
                                qkv_b[2 * C + 64 * h1:2 * C + 64 * h1 + 64]])
        bv = np.ascontiguousarray(
            np.broadcast_to(bvrow[None, :], (128, 128))).astype(bf)
        pwT = np.ascontiguousarray(proj_w[:, 128 * hp:128 * hp + 128].T)
        pw2 = np.ascontiguousarray(
            np.concatenate([pwT[0:64], pwT[64:128]], 1)).astype(bf)
        in_maps.append(dict(xt=xt, wqk=wqk, bqk=bqk, wv=wv, bv=bv, pw2=pw2))
    return in_maps


def _gather(results, proj_b):
    proj_b = np.asarray(proj_b, np.float32)
    out = np.empty((B, N, C), np.float32)
    for b in range(B):
        acc = results[4 * b]["out"].astype(np.float32).copy()
        for hp in range(1, 4):
            acc += results[4 * b + hp]["out"]
        out[b] = acc + proj_b[None, :]
    return out


def _run(x, qkv_w, qkv_b, proj_w, proj_b, trace=False, tmpdir=None):
    from concourse import bass_utils
    nc = _get_nc()
    in_maps = _make_in_maps(x, qkv_w, qkv_b, proj_w)
    res = bass_utils.run_bass_kernel_spmd(
        nc, in_maps, core_ids=list(range(8)), trace=trace, tmpdir=tmpdir)
    return _gather(res.results, proj_b), res


def kernel(x, qkv_w, qkv_b, proj_w, proj_b):
    out, _ = _run(x, qkv_w, qkv_b, proj_w, proj_b, trace=False)
    return out
